# revision 4
# baseline (speedup 1.0000x reference)
"""Embedding lookup (mixed const/trainable tables) on 8 Trainium2 NeuronCores.

Problem (full shapes, fp32):
    X          [524288, 128]   const table (only rows with const_mask==1 are read)
    const_mask [524288]        1 = const row (read from X), 0 = trainable row
    weight     [262144, 128]   trainable table, indexed by rank among mask==0 rows
    index      [262144]        lookup ids into the 524288-row id space
    out        [262144, 128]   out[i] = X[index[i]] if const else weight[var_pos[index[i]]]

Strategy (model parallel):
    - Host compacts X to its const rows (Xe) so both tables have 262144 rows.
    - Both tables are row-sharded over the 8 cores: 32768 rows/core/table
      (32768 so local row ids fit the int16 index format of dma_gather).
    - Host routes each lookup to the owning (core, table) bucket, builds the
      wrapped int16 index tiles, and scatters the gathered rows back into the
      full output afterwards.
    - Device kernel per core: dma_gather (GPSIMD SWDGE) table rows from HBM
      into SBUF tiles, write tiles back to a contiguous DRAM output buffer via
      HWDGE, double-buffered.
"""

import numpy as np

import concourse.bass as bass
import concourse.bacc as bacc
import concourse.mybir as mybir
from concourse.bass_utils import run_bass_kernel_spmd
from concourse.library_config import mlp

NCORES = 8
D = 128            # feature dim (fp32) -> 512B rows
SH = 32768         # table rows per core per table (int16 gather index limit)
CHUNK = 4352       # indices per dma_gather call
TCH = CHUNK // 128   # 34 tile columns per chunk
NCHUNK = 4           # chunks per bucket
CAP = CHUNK * NCHUNK  # 17408 bucket capacity (mean load 16384, 8 sigma margin)
TTOT = CAP // 128    # 136 tile columns per bucket
NBUF = 4             # SBUF tile double buffering depth

_prog_cache = {}
LAST = {}  # debug/profiling introspection for test harnesses


def _build_program():
    """Per-core SPMD bass program: gather CAP rows from each of two tables."""
    nc = bacc.Bacc("TRN2", target_bir_lowering=False)

    tabs = [
        nc.dram_tensor("tabX", [SH, D], mybir.dt.float32, kind="ExternalInput"),
        nc.dram_tensor("tabW", [SH, D], mybir.dt.float32, kind="ExternalInput"),
    ]
    idxs = [
        nc.dram_tensor("idxX", [128, CAP // 16], mybir.dt.int16, kind="ExternalInput"),
        nc.dram_tensor("idxW", [128, CAP // 16], mybir.dt.int16, kind="ExternalInput"),
    ]
    outs = [
        nc.dram_tensor("outX", [128, TTOT, D], mybir.dt.float32, kind="ExternalOutput"),
        nc.dram_tensor("outW", [128, TTOT, D], mybir.dt.float32, kind="ExternalOutput"),
    ]

    K = 2 * NCHUNK  # total chunks
    from contextlib import ExitStack

    with ExitStack() as ctx:
        block = ctx.enter_context(nc.Block())
        idx_sb = [
            ctx.enter_context(
                nc.sbuf_tensor(f"idx_sb{i}", [128, CAP // 16], mybir.dt.int16)
            )
            for i in range(2)
        ]
        tiles = [
            ctx.enter_context(
                nc.sbuf_tensor(f"tile{i}", [128, TCH, D], mybir.dt.float32)
            )
            for i in range(NBUF)
        ]
        io = ctx.enter_context(nc.semaphore("io"))
        gsem = [ctx.enter_context(nc.semaphore(f"g{k}")) for k in range(K)]
        wsem = [ctx.enter_context(nc.semaphore(f"w{k}")) for k in range(K)]

        @block.gpsimd
        def _(g: bass.BassGpSimd):
            g.load_library(mlp)
            g.dma_start(idx_sb[0][:], idxs[0][:]).then_inc(io, 16)
            g.dma_start(idx_sb[1][:], idxs[1][:]).then_inc(io, 16)
            g.wait_ge(io, 32)
            for k in range(K):
                b, j = divmod(k, NCHUNK)
                if k >= NBUF:
                    # wait until the tile buffer's previous contents are written out
                    g.wait_ge(wsem[k - NBUF], 16)
                g.dma_gather(
                    tiles[k % NBUF][:],
                    tabs[b][:],
                    idx_sb[b][:, j * (CHUNK // 16) : (j + 1) * (CHUNK // 16)],
                    CHUNK,
                    CHUNK,
                    D,
                    # single_packet packs each engine's descriptors into one
                    # packet, which is limited to 64 descriptors -> only works
                    # for num_idxs <= 1024. Rows are 512B so packet concat
                    # wouldn't help anyway.
                    single_packet=False,
                ).then_inc(gsem[k], 16)

        @block.sync
        def _(s: bass.BassEngine):
            for k in range(K):
                b, j = divmod(k, NCHUNK)
                s.wait_ge(gsem[k], 16)
                s.dma_start(
                    outs[b][:, j * TCH : (j + 1) * TCH, :], tiles[k % NBUF][:]
                ).then_inc(wsem[k], 16)
            for k in range(K):
                s.wait_ge(wsem[k], 16)

    nc.compile()
    return nc


def get_program():
    if "nc" not in _prog_cache:
        _prog_cache["nc"] = _build_program()
    return _prog_cache["nc"]


def _slot_rows():
    """Flattened [128*TTOT, D] device-buffer row for each bucket list slot."""
    j = np.arange(CAP, dtype=np.int64)
    return (j % 128) * TTOT + (j // CHUNK) * TCH + (j % CHUNK) // 128


def _route(cm, idx, n_weight_rows):
    """Compute per-lookup (bucket, local row) routing.

    Returns (order, counts, local16, const_ids):
      order     argsort by bucket (stable) — original output row per sorted item
      counts    [16] items per bucket (slot*8 + core)
      local16   int16 local table row per sorted item
      const_ids row ids of X that form the compacted const table
    """
    const_rank = np.cumsum(cm) - 1
    var_pos = np.clip(np.cumsum(1 - cm) - 1, 0, n_weight_rows - 1)
    isc = cm[idx] > 0
    r = np.where(isc, const_rank[idx], var_pos[idx])
    bucket = (~isc).astype(np.int64) * NCORES + (r >> 15)
    order = np.argsort(bucket, kind="stable")
    counts = np.bincount(bucket, minlength=2 * NCORES)
    local16 = (r[order] & (SH - 1)).astype(np.int16)
    const_ids = np.flatnonzero(cm > 0)
    return order, counts, local16, const_ids


def _wrap_idx(seg):
    """Pack a bucket's local rows into the [128, CAP/16] wrapped+replicated
    int16 layout dma_gather expects (idx j at partition j%16, col j//16,
    replicated for the 8 Q7 cores)."""
    pad = np.zeros(CAP, np.int16)
    pad[: seg.size] = seg
    wrapped = pad.reshape(CAP // 16, 16).T  # [16, CAP/16]
    return np.ascontiguousarray(np.tile(wrapped, (8, 1)))


def _kernel_numpy(X, cm, weight, idx):
    """Host fallback (used only if structural assumptions break)."""
    var_pos = np.clip(np.cumsum(1 - cm) - 1, 0, weight.shape[0] - 1)
    isc = cm[idx] > 0
    out = np.where(isc[:, None], X[idx], weight[var_pos[idx]])
    return out.astype(np.float32)


def kernel(X, const_mask, weight, index):
    X = np.ascontiguousarray(np.asarray(X), dtype=np.float32)
    weight = np.ascontiguousarray(np.asarray(weight), dtype=np.float32)
    cm = np.asarray(const_mask).astype(np.int64)
    idx = np.asarray(index).astype(np.int64)
    M = idx.shape[0]

    order, counts, local16, const_ids = _route(cm, idx, weight.shape[0])

    structural_ok = (
        X.shape == (524288, 128)
        and weight.shape == (262144, 128)
        and const_ids.size == NCORES * SH
        and weight.shape[0] == NCORES * SH
        and counts.max() <= CAP
    )
    if not structural_ok:
        return _kernel_numpy(X, cm, weight, idx)

    Xe = X[const_ids]  # compacted const table [262144, 128]

    starts = np.concatenate([[0], np.cumsum(counts)])
    in_maps = []
    for c in range(NCORES):
        im = {
            "tabX": Xe[c * SH : (c + 1) * SH],
            "tabW": weight[c * SH : (c + 1) * SH],
        }
        for s, nm in ((0, "idxX"), (1, "idxW")):
            b = s * NCORES + c
            im[nm] = _wrap_idx(local16[starts[b] : starts[b + 1]])
        in_maps.append(im)

    nc = get_program()
    res = run_bass_kernel_spmd(nc, in_maps, core_ids=list(range(NCORES)))
    LAST["res"] = res

    slotrow = _slot_rows()
    out = np.empty((M, D), np.float32)
    for c in range(NCORES):
        for s, nm in ((0, "outX"), (1, "outW")):
            b = s * NCORES + c
            n_b = counts[b]
            if n_b == 0:
                continue
            buf = res.results[c][nm].reshape(128 * TTOT, D)
            out[order[starts[b] : starts[b + 1]]] = buf[slotrow[:n_b]]
    return out


# revision 7
# speedup vs baseline: 1.1017x; 1.1017x over previous
"""Embedding lookup (mixed const/trainable tables) on 8 Trainium2 NeuronCores.

Problem (full shapes, fp32):
    X          [524288, 128]   const table (only rows with const_mask==1 are read)
    const_mask [524288]        1 = const row (read from X), 0 = trainable row
    weight     [262144, 128]   trainable table, indexed by rank among mask==0 rows
    index      [262144]        lookup ids into the 524288-row id space
    out        [262144, 128]   out[i] = X[index[i]] if const else weight[var_pos[index[i]]]

Strategy (model parallel):
    - Host compacts X to its const rows (Xe) so both tables have 262144 rows.
    - Both tables are row-sharded over the 8 cores: 32768 rows/core/table
      (32768 so local row ids fit the int16 index format of dma_gather).
    - Host routes each lookup to the owning (core, table) bucket, builds the
      wrapped int16 index tiles, and scatters the gathered rows back into the
      full output afterwards.
    - Device kernel per core: dma_gather (GPSIMD SWDGE) table rows from HBM
      into SBUF tiles, write tiles back to a contiguous DRAM output buffer via
      HWDGE, double-buffered.
"""

import numpy as np

import concourse.bass as bass
import concourse.bacc as bacc
import concourse.mybir as mybir
from concourse.bass_utils import run_bass_kernel_spmd
from concourse.library_config import mlp

NCORES = 8
D = 128            # feature dim (fp32) -> 512B rows
SH = 32768         # table rows per core per table (int16 gather index limit)
CHUNK = 4480       # indices per dma_gather call
TCH = CHUNK // 128   # 35 tile columns per chunk
NCHUNK = 3           # chunks per bucket
# Capacity is for DISTINCT rows per bucket: 16384 mean lookups hit
# 32768*(1-e^-0.5) ~= 12896 distinct rows; 13440 is ~7 sigma above.
CAP = CHUNK * NCHUNK  # 13440
TTOT = CAP // 128    # 105 tile columns per bucket
NBUF = 4             # SBUF tile double buffering depth

_prog_cache = {}
LAST = {}  # debug/profiling introspection for test harnesses


def _build_program():
    """Per-core SPMD bass program: gather CAP rows from each of two tables."""
    nc = bacc.Bacc("TRN2", target_bir_lowering=False)

    tabs = [
        nc.dram_tensor("tabX", [SH, D], mybir.dt.float32, kind="ExternalInput"),
        nc.dram_tensor("tabW", [SH, D], mybir.dt.float32, kind="ExternalInput"),
    ]
    idxs = [
        nc.dram_tensor("idxX", [128, CAP // 16], mybir.dt.int16, kind="ExternalInput"),
        nc.dram_tensor("idxW", [128, CAP // 16], mybir.dt.int16, kind="ExternalInput"),
    ]
    outs = [
        nc.dram_tensor("outX", [128, TTOT, D], mybir.dt.float32, kind="ExternalOutput"),
        nc.dram_tensor("outW", [128, TTOT, D], mybir.dt.float32, kind="ExternalOutput"),
    ]

    K = 2 * NCHUNK  # total chunks
    from contextlib import ExitStack

    with ExitStack() as ctx:
        block = ctx.enter_context(nc.Block())
        idx_sb = [
            ctx.enter_context(
                nc.sbuf_tensor(f"idx_sb{i}", [128, CAP // 16], mybir.dt.int16)
            )
            for i in range(2)
        ]
        tiles = [
            ctx.enter_context(
                nc.sbuf_tensor(f"tile{i}", [128, TCH, D], mybir.dt.float32)
            )
            for i in range(NBUF)
        ]
        io = ctx.enter_context(nc.semaphore("io"))
        gsem = [ctx.enter_context(nc.semaphore(f"g{k}")) for k in range(K)]
        wsem = [ctx.enter_context(nc.semaphore(f"w{k}")) for k in range(K)]

        @block.gpsimd
        def _(g: bass.BassGpSimd):
            g.load_library(mlp)
            g.dma_start(idx_sb[0][:], idxs[0][:]).then_inc(io, 16)
            g.dma_start(idx_sb[1][:], idxs[1][:]).then_inc(io, 16)
            g.wait_ge(io, 32)
            for k in range(K):
                b, j = divmod(k, NCHUNK)
                if k >= NBUF:
                    # wait until the tile buffer's previous contents are written out
                    g.wait_ge(wsem[k - NBUF], 16)
                g.dma_gather(
                    tiles[k % NBUF][:],
                    tabs[b][:],
                    idx_sb[b][:, j * (CHUNK // 16) : (j + 1) * (CHUNK // 16)],
                    CHUNK,
                    CHUNK,
                    D,
                    # single_packet packs each engine's descriptors into one
                    # packet, which is limited to 64 descriptors -> only works
                    # for num_idxs <= 1024. Rows are 512B so packet concat
                    # wouldn't help anyway.
                    single_packet=False,
                ).then_inc(gsem[k], 16)

        @block.sync
        def _(s: bass.BassEngine):
            for k in range(K):
                b, j = divmod(k, NCHUNK)
                s.wait_ge(gsem[k], 16)
                s.dma_start(
                    outs[b][:, j * TCH : (j + 1) * TCH, :], tiles[k % NBUF][:]
                ).then_inc(wsem[k], 16)
            for k in range(K):
                s.wait_ge(wsem[k], 16)

    nc.compile()
    return nc


def get_program():
    if "nc" not in _prog_cache:
        _prog_cache["nc"] = _build_program()
    return _prog_cache["nc"]


def _slot_rows():
    """Flattened [128*TTOT, D] device-buffer row for each bucket list slot."""
    j = np.arange(CAP, dtype=np.int64)
    return (j % 128) * TTOT + (j // CHUNK) * TCH + (j % CHUNK) // 128


def _route(cm, idx, n_weight_rows):
    """Compute deduplicated (bucket, local row) routing.

    Each distinct (table, row) pair is gathered once; duplicates expand in the
    host-side scatter for free (Q7 descriptor generation cost is per gathered
    row, so dedupe is a direct ~20% win).

    Returns (ulocal, counts, inv, const_ids):
      ulocal    int16 local table row per distinct slot, bucket-major and
                sorted ascending within each bucket
      counts    [16] distinct rows per bucket (bucket = slot*8 + core)
      inv       per-lookup index into the distinct-slot space
      const_ids row ids of X that form the compacted const table
    """
    const_rank = np.cumsum(cm) - 1
    var_pos = np.clip(np.cumsum(1 - cm) - 1, 0, n_weight_rows - 1)
    isc = cm[idx] > 0
    r = np.where(isc, const_rank[idx], var_pos[idx])
    bucket = (~isc).astype(np.int64) * NCORES + (r >> 15)
    key = bucket * SH + (r & (SH - 1))
    uniq, inv = np.unique(key, return_inverse=True)
    counts = np.bincount(uniq // SH, minlength=2 * NCORES)
    ulocal = (uniq % SH).astype(np.int16)
    const_ids = np.flatnonzero(cm > 0)
    return ulocal, counts, inv, const_ids


def _wrap_idx(seg):
    """Pack a bucket's local rows into the [128, CAP/16] wrapped+replicated
    int16 layout dma_gather expects (idx j at partition j%16, col j//16,
    replicated for the 8 Q7 cores)."""
    pad = np.zeros(CAP, np.int16)
    pad[: seg.size] = seg
    wrapped = pad.reshape(CAP // 16, 16).T  # [16, CAP/16]
    return np.ascontiguousarray(np.tile(wrapped, (8, 1)))


def _kernel_numpy(X, cm, weight, idx):
    """Host fallback (used only if structural assumptions break)."""
    var_pos = np.clip(np.cumsum(1 - cm) - 1, 0, weight.shape[0] - 1)
    isc = cm[idx] > 0
    out = np.where(isc[:, None], X[idx], weight[var_pos[idx]])
    return out.astype(np.float32)


def kernel(X, const_mask, weight, index):
    X = np.ascontiguousarray(np.asarray(X), dtype=np.float32)
    weight = np.ascontiguousarray(np.asarray(weight), dtype=np.float32)
    cm = np.asarray(const_mask).astype(np.int64)
    idx = np.asarray(index).astype(np.int64)
    M = idx.shape[0]

    ulocal, counts, inv, const_ids = _route(cm, idx, weight.shape[0])

    structural_ok = (
        X.shape == (524288, 128)
        and weight.shape == (262144, 128)
        and const_ids.size == NCORES * SH
        and weight.shape[0] == NCORES * SH
        and counts.max() <= CAP
    )
    if not structural_ok:
        return _kernel_numpy(X, cm, weight, idx)

    Xe = X[const_ids]  # compacted const table [262144, 128]

    starts = np.concatenate([[0], np.cumsum(counts)])
    in_maps = []
    for c in range(NCORES):
        im = {
            "tabX": Xe[c * SH : (c + 1) * SH],
            "tabW": weight[c * SH : (c + 1) * SH],
        }
        for s, nm in ((0, "idxX"), (1, "idxW")):
            b = s * NCORES + c
            im[nm] = _wrap_idx(ulocal[starts[b] : starts[b + 1]])
        in_maps.append(im)

    nc = get_program()
    res = run_bass_kernel_spmd(nc, in_maps, core_ids=list(range(NCORES)))
    LAST["res"] = res

    # distinct rows in bucket-major order, then expand duplicates per lookup
    slotrow = _slot_rows()
    allrows = np.empty((ulocal.size, D), np.float32)
    for c in range(NCORES):
        for s, nm in ((0, "outX"), (1, "outW")):
            b = s * NCORES + c
            n_b = counts[b]
            if n_b == 0:
                continue
            buf = res.results[c][nm].reshape(128 * TTOT, D)
            allrows[starts[b] : starts[b + 1]] = buf[slotrow[:n_b]]
    return allrows[inv]


# revision 8
# speedup vs baseline: 1.2100x; 1.0983x over previous
"""Embedding lookup (mixed const/trainable tables) on 8 Trainium2 NeuronCores.

Problem (full shapes, fp32):
    X          [524288, 128]   const table (only rows with const_mask==1 are read)
    const_mask [524288]        1 = const row (read from X), 0 = trainable row
    weight     [262144, 128]   trainable table, indexed by rank among mask==0 rows
    index      [262144]        lookup ids into the 524288-row id space
    out        [262144, 128]   out[i] = X[index[i]] if const else weight[var_pos[index[i]]]

Strategy (model parallel, deduplicated, pair-merged):
    - Host compacts X to its const rows (Xe) so both tables have 262144 rows;
      both are row-sharded over the 8 cores (32768 rows/core/table so local
      row ids fit dma_gather's int16 index format).
    - Each lookup routes to the owning (core, table) bucket. Each bucket is
      DEDUPLICATED (a distinct row is gathered once; duplicates expand in the
      host-side scatter) because GPSIMD descriptor generation (~8ns/row) is
      the kernel bottleneck, then split into
        * aligned PAIRS: rows (2k, 2k+1) both needed -> one 1024B descriptor
        * SINGLES: remaining rows -> one 512B descriptor
      cutting descriptor count another ~20%.
    - Actual pair/single counts are passed per core in a tiny `cnts` tensor
      and loaded into Q7 registers (num_idxs_reg), so the -1 tail padding in
      the index tiles costs nothing.
    - Device kernel per core: 4 dma_gather (GPSIMD SWDGE) HBM->SBUF streams,
      each followed by one large HWDGE write SBUF->HBM, overlapped.
    - Host scatters the gathered distinct rows back to all lookup positions.
"""

import numpy as np

import concourse.bass as bass
import concourse.bacc as bacc
import concourse.mybir as mybir
from concourse.bass_utils import run_bass_kernel_spmd
from concourse.library_config import mlp

NCORES = 8
D = 128             # feature dim (fp32) -> 512B rows
SH = 32768          # table rows per core per table (int16 gather index limit)

# Distinct rows per bucket: 16384 mean lookups hit 32768*(1-e^-0.5) ~= 12896
# distinct rows, of which ~5078 form ~2539 aligned pairs and ~7818 are
# singles. Capacities are ~7-9 sigma above those means.
CAP_P = 2944        # pair slots per bucket   (TP = 23 tile columns of 1024B)
CAP_S = 8320        # single slots per bucket (TS = 65 tile columns of 512B)
TP = CAP_P // 128
TS = CAP_S // 128

# stream order: (bucket, kind) -> gather issue order; last write is the tail
# so end on a pair stream (smallest write).
STREAMS = (("X", "S"), ("X", "P"), ("W", "S"), ("W", "P"))

_prog_cache = {}
LAST = {}  # debug/profiling introspection for test harnesses


def _stream_dims(kind):
    if kind == "P":
        return CAP_P, TP, 2 * D  # 1024B elements (row pairs)
    return CAP_S, TS, D          # 512B elements


def _build_program():
    """Per-core SPMD bass program: 4 exact-count gather streams + 4 writes."""
    nc = bacc.Bacc("TRN2", target_bir_lowering=False)

    tabs = {
        "X": nc.dram_tensor("tabX", [SH, D], mybir.dt.float32, kind="ExternalInput"),
        "W": nc.dram_tensor("tabW", [SH, D], mybir.dt.float32, kind="ExternalInput"),
    }
    idxs, outs = {}, {}
    for b, k in STREAMS:
        cap, t, elem = _stream_dims(k)
        idxs[b + k] = nc.dram_tensor(
            f"idx{b}{k}", [128, cap // 16], mybir.dt.int16, kind="ExternalInput"
        )
        outs[b + k] = nc.dram_tensor(
            f"out{b}{k}", [128, t, elem], mybir.dt.float32, kind="ExternalOutput"
        )
    cnts = nc.dram_tensor("cnts", [128, 4], mybir.dt.int32, kind="ExternalInput")

    from contextlib import ExitStack

    with ExitStack() as ctx:
        block = ctx.enter_context(nc.Block())
        idx_sb, tiles, gsem, wsem = {}, {}, {}, {}
        for b, k in STREAMS:
            cap, t, elem = _stream_dims(k)
            idx_sb[b + k] = ctx.enter_context(
                nc.sbuf_tensor(f"isb{b}{k}", [128, cap // 16], mybir.dt.int16)
            )
            tiles[b + k] = ctx.enter_context(
                nc.sbuf_tensor(f"tile{b}{k}", [128, t, elem], mybir.dt.float32)
            )
            gsem[b + k] = ctx.enter_context(nc.semaphore(f"g{b}{k}"))
            wsem[b + k] = ctx.enter_context(nc.semaphore(f"w{b}{k}"))
        csb = ctx.enter_context(nc.sbuf_tensor("csb", [128, 4], mybir.dt.int32))
        io = ctx.enter_context(nc.semaphore("io"))

        @block.gpsimd
        def _(g: bass.BassGpSimd):
            g.load_library(mlp)
            n_in = 0
            for b, k in STREAMS:
                g.dma_start(idx_sb[b + k][:], idxs[b + k][:]).then_inc(io, 16)
                n_in += 16
            g.dma_start(csb[:], cnts[:]).then_inc(io, 16)
            g.wait_ge(io, n_in + 16)
            from contextlib import ExitStack as ES

            with ES() as rctx:
                regs = {
                    b + k: rctx.enter_context(g.register(f"r{b}{k}"))
                    for b, k in STREAMS
                }
                for i, (b, k) in enumerate(STREAMS):
                    g.reg_load(regs[b + k], csb[0:1, i : i + 1])
                for b, k in STREAMS:
                    cap, t, elem = _stream_dims(k)
                    src = tabs[b][:]
                    if k == "P":
                        src = src.rearrange("(a b) d -> a (b d)", b=2)
                    g.dma_gather(
                        tiles[b + k][:],
                        src,
                        idx_sb[b + k][:],
                        cap,
                        regs[b + k],
                        elem,
                        single_packet=False,
                    ).then_inc(gsem[b + k], 16)

        @block.sync
        def _(s: bass.BassEngine):
            for b, k in STREAMS:
                s.wait_ge(gsem[b + k], 16)
                s.dma_start(outs[b + k][:], tiles[b + k][:]).then_inc(wsem[b + k], 16)
            for b, k in STREAMS:
                s.wait_ge(wsem[b + k], 16)

    nc.compile()
    return nc


def get_program():
    if "nc" not in _prog_cache:
        _prog_cache["nc"] = _build_program()
    return _prog_cache["nc"]


def _slot_rows(cap, t):
    """Flattened [128*t, elem] device-buffer row for each gather list slot."""
    j = np.arange(cap, dtype=np.int64)
    return (j % 128) * t + j // 128


def _wrap_idx(seg, cap):
    """Pack a stream's int16 ids into the [128, cap/16] wrapped+replicated
    layout dma_gather expects (idx j at partition j%16, col j//16, replicated
    for the 8 Q7 cores), -1 padded."""
    pad = np.full(cap, -1, np.int16)
    pad[: seg.size] = seg
    wrapped = pad.reshape(cap // 16, 16).T  # [16, cap/16]
    return np.ascontiguousarray(np.tile(wrapped, (8, 1)))


def _route(cm, idx, n_weight_rows):
    """Deduplicated (bucket, local row) routing.

    Returns (ulocal, counts, inv, const_ids):
      ulocal    local table row per distinct slot, bucket-major, sorted
      counts    [16] distinct rows per bucket (bucket = slot*8 + core)
      inv       per-lookup index into the distinct-slot space
      const_ids row ids of X that form the compacted const table
    """
    const_rank = np.cumsum(cm) - 1
    var_pos = np.clip(np.cumsum(1 - cm) - 1, 0, n_weight_rows - 1)
    isc = cm[idx] > 0
    r = np.where(isc, const_rank[idx], var_pos[idx])
    bucket = (~isc).astype(np.int64) * NCORES + (r >> 15)
    key = bucket * SH + (r & (SH - 1))
    uniq, inv = np.unique(key, return_inverse=True)
    counts = np.bincount(uniq // SH, minlength=2 * NCORES)
    ulocal = uniq % SH
    const_ids = np.flatnonzero(cm > 0)
    return ulocal, counts, inv, const_ids


def _split_pairs(u):
    """Split sorted distinct rows into aligned pairs and singles.

    Returns (pairs, singles, ispaired, pos):
      pairs    sorted pair ids k where rows 2k and 2k+1 are both in u
      singles  sorted rows of u not covered by pairs
      ispaired bool per element of u
      pos      per element of u: its index within pairs (paired) or singles
    """
    pid = u >> 1
    eq = pid[:-1] == pid[1:]  # u[i], u[i+1] form an aligned pair
    ispaired = np.zeros(u.size, bool)
    ispaired[:-1] |= eq
    ispaired[1:] |= eq
    pairs = pid[:-1][eq]
    singles = u[~ispaired]
    pos = np.empty(u.size, np.int64)
    pos[ispaired] = np.searchsorted(pairs, pid[ispaired])
    pos[~ispaired] = np.arange(singles.size)
    return pairs, singles, ispaired, pos


def _kernel_numpy(X, cm, weight, idx):
    """Host fallback (used only if structural assumptions break)."""
    var_pos = np.clip(np.cumsum(1 - cm) - 1, 0, weight.shape[0] - 1)
    isc = cm[idx] > 0
    out = np.where(isc[:, None], X[idx], weight[var_pos[idx]])
    return out.astype(np.float32)


def kernel(X, const_mask, weight, index):
    X = np.ascontiguousarray(np.asarray(X), dtype=np.float32)
    weight = np.ascontiguousarray(np.asarray(weight), dtype=np.float32)
    cm = np.asarray(const_mask).astype(np.int64)
    idx = np.asarray(index).astype(np.int64)
    M = idx.shape[0]

    ulocal, counts, inv, const_ids = _route(cm, idx, weight.shape[0])
    starts = np.concatenate([[0], np.cumsum(counts)])

    # per-bucket pair/single split
    splits = [_split_pairs(ulocal[starts[b] : starts[b + 1]]) for b in range(16)]

    structural_ok = (
        X.shape == (524288, 128)
        and weight.shape == (262144, 128)
        and const_ids.size == NCORES * SH
        and weight.shape[0] == NCORES * SH
        and all(1 <= p.size <= CAP_P and 1 <= s.size <= CAP_S for p, s, _, _ in splits)
    )
    if not structural_ok:
        return _kernel_numpy(X, cm, weight, idx)

    Xe = X[const_ids]  # compacted const table [262144, 128]

    in_maps = []
    for c in range(NCORES):
        im = {
            "tabX": Xe[c * SH : (c + 1) * SH],
            "tabW": weight[c * SH : (c + 1) * SH],
        }
        cvec = np.empty(4, np.int32)
        for i, (b, k) in enumerate(STREAMS):
            bkt = (0 if b == "X" else NCORES) + c
            pairs, singles, _, _ = splits[bkt]
            ids = pairs if k == "P" else singles
            cap = _stream_dims(k)[0]
            im[f"idx{b}{k}"] = _wrap_idx(ids.astype(np.int16), cap)
            cvec[i] = ids.size
        im["cnts"] = np.ascontiguousarray(np.tile(cvec, (128, 1)))
        in_maps.append(im)

    nc = get_program()
    res = run_bass_kernel_spmd(nc, in_maps, core_ids=list(range(NCORES)))
    LAST["res"] = res

    # reassemble: distinct rows bucket-major, then expand duplicates per lookup
    srow_p = _slot_rows(CAP_P, TP)
    srow_s = _slot_rows(CAP_S, TS)
    allrows = np.empty((ulocal.size, D), np.float32)
    for c in range(NCORES):
        for b in ("X", "W"):
            bkt = (0 if b == "X" else NCORES) + c
            pairs, singles, ispaired, pos = splits[bkt]
            seg = slice(starts[bkt], starts[bkt + 1])
            u = ulocal[seg]
            arr = np.empty((u.size, D), np.float32)
            bufP = res.results[c][f"out{b}P"].reshape(128 * TP * 2, D)
            bufS = res.results[c][f"out{b}S"].reshape(128 * TS, D)
            arr[ispaired] = bufP[srow_p[pos[ispaired]] * 2 + (u[ispaired] & 1)]
            arr[~ispaired] = bufS[srow_s[pos[~ispaired]]]
            allrows[seg] = arr
    return allrows[inv]


# revision 10
# speedup vs baseline: 1.4215x; 1.1748x over previous
"""Embedding lookup (mixed const/trainable tables) on 8 Trainium2 NeuronCores.

Problem (full shapes, fp32):
    X          [524288, 128]   const table (only rows with const_mask==1 are read)
    const_mask [524288]        1 = const row (read from X), 0 = trainable row
    weight     [262144, 128]   trainable table, indexed by rank among mask==0 rows
    index      [262144]        lookup ids into the 524288-row id space
    out        [262144, 128]   out[i] = X[index[i]] if const else weight[var_pos[index[i]]]

Strategy (model parallel, deduplicated, pair-merged):
    - Host compacts X to its const rows (Xe) so both tables have 262144 rows;
      both are row-sharded over the 8 cores (32768 rows/core/table so local
      row ids fit dma_gather's int16 index format).
    - Each lookup routes to the owning (core, table) bucket. Each bucket is
      DEDUPLICATED (a distinct row is gathered once; duplicates expand in the
      host-side scatter) because GPSIMD descriptor generation (~8ns/row) is
      the kernel bottleneck, then split into
        * aligned PAIRS: rows (2k, 2k+1) both needed -> one 1024B descriptor
        * SINGLES: remaining rows -> one 512B descriptor
      cutting descriptor count another ~20%.
    - Actual pair/single counts are passed per core in a tiny `cnts` tensor
      and loaded into Q7 registers (num_idxs_reg), so the -1 tail padding in
      the index tiles costs nothing.
    - Device kernel per core: 4 dma_gather (GPSIMD SWDGE) HBM->SBUF streams,
      each followed by one large HWDGE write SBUF->HBM, overlapped.
    - Host scatters the gathered distinct rows back to all lookup positions.
"""

import numpy as np

import concourse.bass as bass
import concourse.bacc as bacc
import concourse.mybir as mybir
from concourse.bass_utils import run_bass_kernel_spmd
from concourse.library_config import mlp

NCORES = 8
D = 128             # feature dim (fp32) -> 512B rows
SH = 32768          # table rows per core per table (int16 gather index limit)

# Distinct rows per bucket: 16384 mean lookups hit 32768*(1-e^-0.5) ~= 12896
# distinct rows, of which ~5078 form ~2539 aligned pairs and ~7818 are
# singles. Capacities are ~7-9 sigma above those means.
CAP_P = 2944        # pair slots per bucket   (TP = 23 tile columns of 1024B)
CAP_S = 8320        # single slots per bucket (TS = 65 tile columns of 512B)
TP = CAP_P // 128
TS = CAP_S // 128

# stream order: (bucket, kind) -> gather issue order; last write is the tail
# so end on a pair stream (smallest write).
STREAMS = (("X", "S"), ("X", "P"), ("W", "S"), ("W", "P"))

_prog_cache = {}
LAST = {}  # debug/profiling introspection for test harnesses


def _stream_dims(kind):
    if kind == "P":
        return CAP_P, TP, 2 * D  # 1024B elements (row pairs)
    return CAP_S, TS, D          # 512B elements


def _build_program():
    """Per-core SPMD bass program: 4 exact-count gather streams + 4 writes."""
    nc = bacc.Bacc("TRN2", target_bir_lowering=False)

    tabs = {
        "X": nc.dram_tensor("tabX", [SH, D], mybir.dt.float32, kind="ExternalInput"),
        "W": nc.dram_tensor("tabW", [SH, D], mybir.dt.float32, kind="ExternalInput"),
    }
    idxs, outs = {}, {}
    for b, k in STREAMS:
        cap, t, elem = _stream_dims(k)
        idxs[b + k] = nc.dram_tensor(
            f"idx{b}{k}", [128, cap // 16], mybir.dt.int16, kind="ExternalInput"
        )
        outs[b + k] = nc.dram_tensor(
            f"out{b}{k}", [128, t, elem], mybir.dt.float32, kind="ExternalOutput"
        )
    cnts = nc.dram_tensor("cnts", [128, 4], mybir.dt.int32, kind="ExternalInput")

    from contextlib import ExitStack

    with ExitStack() as ctx:
        block = ctx.enter_context(nc.Block())
        idx_sb, tiles, gsem, wsem = {}, {}, {}, {}
        for b, k in STREAMS:
            cap, t, elem = _stream_dims(k)
            idx_sb[b + k] = ctx.enter_context(
                nc.sbuf_tensor(f"isb{b}{k}", [128, cap // 16], mybir.dt.int16)
            )
            tiles[b + k] = ctx.enter_context(
                nc.sbuf_tensor(f"tile{b}{k}", [128, t, elem], mybir.dt.float32)
            )
            gsem[b + k] = ctx.enter_context(nc.semaphore(f"g{b}{k}"))
            wsem[b + k] = ctx.enter_context(nc.semaphore(f"w{b}{k}"))
        csb = ctx.enter_context(nc.sbuf_tensor("csb", [128, 4], mybir.dt.int32))
        io = ctx.enter_context(nc.semaphore("io"))

        @block.gpsimd
        def _(g: bass.BassGpSimd):
            # issue input loads first so the transfers overlap the library
            # reload (the SDMA work needs no Q7 involvement once issued)
            for b, k in STREAMS:
                g.dma_start(idx_sb[b + k][:], idxs[b + k][:]).then_inc(io, 16)
            g.dma_start(csb[:], cnts[:]).then_inc(io, 16)
            g.load_library(mlp)
            g.wait_ge(io, 80)
            from contextlib import ExitStack as ES

            with ES() as rctx:
                regs = {
                    b + k: rctx.enter_context(g.register(f"r{b}{k}"))
                    for b, k in STREAMS
                }
                for i, (b, k) in enumerate(STREAMS):
                    g.reg_load(regs[b + k], csb[0:1, i : i + 1])
                for b, k in STREAMS:
                    cap, t, elem = _stream_dims(k)
                    src = tabs[b][:]
                    if k == "P":
                        src = src.rearrange("(a b) d -> a (b d)", b=2)
                    g.dma_gather(
                        tiles[b + k][:],
                        src,
                        idx_sb[b + k][:],
                        cap,
                        regs[b + k],
                        elem,
                        single_packet=False,
                    ).then_inc(gsem[b + k], 16)

        @block.sync
        def _(s: bass.BassEngine):
            for b, k in STREAMS:
                s.wait_ge(gsem[b + k], 16)
                s.dma_start(outs[b + k][:], tiles[b + k][:]).then_inc(wsem[b + k], 16)
            for b, k in STREAMS:
                s.wait_ge(wsem[b + k], 16)

    nc.compile()
    return nc


def get_program():
    if "nc" not in _prog_cache:
        _prog_cache["nc"] = _build_program()
    return _prog_cache["nc"]


def _slot_rows(cap, t):
    """Flattened [128*t, elem] device-buffer row for each gather list slot."""
    j = np.arange(cap, dtype=np.int64)
    return (j % 128) * t + j // 128


def _wrap_idx(seg, cap):
    """Pack a stream's int16 ids into the [128, cap/16] wrapped+replicated
    layout dma_gather expects (idx j at partition j%16, col j//16, replicated
    for the 8 Q7 cores), -1 padded."""
    pad = np.full(cap, -1, np.int16)
    pad[: seg.size] = seg
    wrapped = pad.reshape(cap // 16, 16).T  # [16, cap/16]
    return np.ascontiguousarray(np.tile(wrapped, (8, 1)))


def _route(cm, idx, n_weight_rows):
    """Deduplicated (bucket, local row) routing.

    Returns (ulocal, counts, inv, const_ids):
      ulocal    local table row per distinct slot, bucket-major, sorted
      counts    [16] distinct rows per bucket (bucket = slot*8 + core)
      inv       per-lookup index into the distinct-slot space
      const_ids row ids of X that form the compacted const table
    """
    const_rank = np.cumsum(cm) - 1
    var_pos = np.clip(np.cumsum(1 - cm) - 1, 0, n_weight_rows - 1)
    isc = cm[idx] > 0
    r = np.where(isc, const_rank[idx], var_pos[idx])
    bucket = (~isc).astype(np.int64) * NCORES + (r >> 15)
    key = bucket * SH + (r & (SH - 1))
    uniq, inv = np.unique(key, return_inverse=True)
    counts = np.bincount(uniq // SH, minlength=2 * NCORES)
    ulocal = uniq % SH
    const_ids = np.flatnonzero(cm > 0)
    return ulocal, counts, inv, const_ids


def _split_pairs(u):
    """Split sorted distinct rows into aligned pairs and singles.

    Returns (pairs, singles, ispaired, pos):
      pairs    sorted pair ids k where rows 2k and 2k+1 are both in u
      singles  sorted rows of u not covered by pairs
      ispaired bool per element of u
      pos      per element of u: its index within pairs (paired) or singles
    """
    pid = u >> 1
    eq = pid[:-1] == pid[1:]  # u[i], u[i+1] form an aligned pair
    ispaired = np.zeros(u.size, bool)
    ispaired[:-1] |= eq
    ispaired[1:] |= eq
    pairs = pid[:-1][eq]
    singles = u[~ispaired]
    pos = np.empty(u.size, np.int64)
    pos[ispaired] = np.searchsorted(pairs, pid[ispaired])
    pos[~ispaired] = np.arange(singles.size)
    return pairs, singles, ispaired, pos


def _kernel_numpy(X, cm, weight, idx):
    """Host fallback (used only if structural assumptions break)."""
    var_pos = np.clip(np.cumsum(1 - cm) - 1, 0, weight.shape[0] - 1)
    isc = cm[idx] > 0
    out = np.where(isc[:, None], X[idx], weight[var_pos[idx]])
    return out.astype(np.float32)


def kernel(X, const_mask, weight, index):
    X = np.ascontiguousarray(np.asarray(X), dtype=np.float32)
    weight = np.ascontiguousarray(np.asarray(weight), dtype=np.float32)
    cm = np.asarray(const_mask).astype(np.int64)
    idx = np.asarray(index).astype(np.int64)
    M = idx.shape[0]

    ulocal, counts, inv, const_ids = _route(cm, idx, weight.shape[0])
    starts = np.concatenate([[0], np.cumsum(counts)])

    # per-bucket pair/single split
    splits = [_split_pairs(ulocal[starts[b] : starts[b + 1]]) for b in range(16)]

    structural_ok = (
        X.shape == (524288, 128)
        and weight.shape == (262144, 128)
        and const_ids.size == NCORES * SH
        and weight.shape[0] == NCORES * SH
        and all(1 <= p.size <= CAP_P and 1 <= s.size <= CAP_S for p, s, _, _ in splits)
    )
    if not structural_ok:
        return _kernel_numpy(X, cm, weight, idx)

    Xe = X[const_ids]  # compacted const table [262144, 128]

    in_maps = []
    for c in range(NCORES):
        im = {
            "tabX": Xe[c * SH : (c + 1) * SH],
            "tabW": weight[c * SH : (c + 1) * SH],
        }
        cvec = np.empty(4, np.int32)
        for i, (b, k) in enumerate(STREAMS):
            bkt = (0 if b == "X" else NCORES) + c
            pairs, singles, _, _ = splits[bkt]
            ids = pairs if k == "P" else singles
            cap = _stream_dims(k)[0]
            im[f"idx{b}{k}"] = _wrap_idx(ids.astype(np.int16), cap)
            cvec[i] = ids.size
        im["cnts"] = np.ascontiguousarray(np.tile(cvec, (128, 1)))
        in_maps.append(im)

    nc = get_program()
    res = run_bass_kernel_spmd(nc, in_maps, core_ids=list(range(NCORES)))
    LAST["res"] = res

    # reassemble: distinct rows bucket-major, then expand duplicates per lookup
    srow_p = _slot_rows(CAP_P, TP)
    srow_s = _slot_rows(CAP_S, TS)
    allrows = np.empty((ulocal.size, D), np.float32)
    for c in range(NCORES):
        for b in ("X", "W"):
            bkt = (0 if b == "X" else NCORES) + c
            pairs, singles, ispaired, pos = splits[bkt]
            seg = slice(starts[bkt], starts[bkt + 1])
            u = ulocal[seg]
            arr = np.empty((u.size, D), np.float32)
            bufP = res.results[c][f"out{b}P"].reshape(128 * TP * 2, D)
            bufS = res.results[c][f"out{b}S"].reshape(128 * TS, D)
            arr[ispaired] = bufP[srow_p[pos[ispaired]] * 2 + (u[ispaired] & 1)]
            arr[~ispaired] = bufS[srow_s[pos[~ispaired]]]
            allrows[seg] = arr
    return allrows[inv]


# revision 11
# speedup vs baseline: 1.4375x; 1.0113x over previous
"""Embedding lookup (mixed const/trainable tables) on 8 Trainium2 NeuronCores.

Problem (full shapes, fp32):
    X          [524288, 128]   const table (only rows with const_mask==1 are read)
    const_mask [524288]        1 = const row (read from X), 0 = trainable row
    weight     [262144, 128]   trainable table, indexed by rank among mask==0 rows
    index      [262144]        lookup ids into the 524288-row id space
    out        [262144, 128]   out[i] = X[index[i]] if const else weight[var_pos[index[i]]]

Strategy (model parallel, deduplicated, pair-merged):
    - Host compacts X to its const rows (Xe) so both tables have 262144 rows;
      both are row-sharded over the 8 cores (32768 rows/core/table so local
      row ids fit dma_gather's int16 index format).
    - Each lookup routes to the owning (core, table) bucket. Each bucket is
      DEDUPLICATED (a distinct row is gathered once; duplicates expand in the
      host-side scatter) because GPSIMD descriptor generation (~8ns/row) is
      the kernel bottleneck, then split into
        * aligned PAIRS: rows (2k, 2k+1) both needed -> one 1024B descriptor
        * SINGLES: remaining rows -> one 512B descriptor
      cutting descriptor count another ~20%.
    - Actual pair/single counts are passed per core in a tiny `cnts` tensor
      and loaded into Q7 registers (num_idxs_reg), so the -1 tail padding in
      the index tiles costs nothing.
    - Device kernel per core: 4 dma_gather (GPSIMD SWDGE) HBM->SBUF streams,
      each followed by one large HWDGE write SBUF->HBM, overlapped.
    - Host scatters the gathered distinct rows back to all lookup positions.
"""

import numpy as np

import concourse.bass as bass
import concourse.bacc as bacc
import concourse.mybir as mybir
from concourse.bass_utils import run_bass_kernel_spmd
from concourse.library_config import mlp

NCORES = 8
D = 128             # feature dim (fp32) -> 512B rows
SH = 32768          # table rows per core per table (int16 gather index limit)

# Distinct rows per bucket: 16384 mean lookups hit 32768*(1-e^-0.5) ~= 12896
# distinct rows, of which ~5078 form ~2539 aligned pairs and ~7818 are
# singles. Capacities are ~7-9 sigma above those means.
CAP_P = 2944        # pair slots per bucket   (TP = 23 tile columns of 1024B)
CAP_S = 8320        # single slots per bucket (TS = 65 tile columns of 512B)
TP = CAP_P // 128
TS = CAP_S // 128

# stream order: (bucket, kind) -> gather issue order; last write is the tail
# so end on a pair stream (smallest write).
STREAMS = (("X", "S"), ("X", "P"), ("W", "S"), ("W", "P"))

_prog_cache = {}
LAST = {}  # debug/profiling introspection for test harnesses


def _stream_dims(kind):
    if kind == "P":
        return CAP_P, TP, 2 * D  # 1024B elements (row pairs)
    return CAP_S, TS, D          # 512B elements


def _build_program():
    """Per-core SPMD bass program: 4 exact-count gather streams + 4 writes."""
    nc = bacc.Bacc("TRN2", target_bir_lowering=False)

    tabs = {
        "X": nc.dram_tensor("tabX", [SH, D], mybir.dt.float32, kind="ExternalInput"),
        "W": nc.dram_tensor("tabW", [SH, D], mybir.dt.float32, kind="ExternalInput"),
    }
    idxs, outs = {}, {}
    for b, k in STREAMS:
        cap, t, elem = _stream_dims(k)
        idxs[b + k] = nc.dram_tensor(
            f"idx{b}{k}", [128, cap // 16], mybir.dt.int16, kind="ExternalInput"
        )
        outs[b + k] = nc.dram_tensor(
            f"out{b}{k}", [128, t, elem], mybir.dt.float32, kind="ExternalOutput"
        )
    cnts = nc.dram_tensor("cnts", [128, 4], mybir.dt.int32, kind="ExternalInput")

    from contextlib import ExitStack

    with ExitStack() as ctx:
        # write-completion sems already guarantee all DMAs retired; skipping
        # the gpsimd dge_drain removes ~10us from the kernel tail
        block = ctx.enter_context(nc.Block(no_gpsimd_drain=True))
        idx_sb, tiles, gsem, wsem = {}, {}, {}, {}
        for b, k in STREAMS:
            cap, t, elem = _stream_dims(k)
            idx_sb[b + k] = ctx.enter_context(
                nc.sbuf_tensor(f"isb{b}{k}", [128, cap // 16], mybir.dt.int16)
            )
            tiles[b + k] = ctx.enter_context(
                nc.sbuf_tensor(f"tile{b}{k}", [128, t, elem], mybir.dt.float32)
            )
            gsem[b + k] = ctx.enter_context(nc.semaphore(f"g{b}{k}"))
            wsem[b + k] = ctx.enter_context(nc.semaphore(f"w{b}{k}"))
        csb = ctx.enter_context(nc.sbuf_tensor("csb", [128, 4], mybir.dt.int32))
        io = ctx.enter_context(nc.semaphore("io"))

        @block.gpsimd
        def _(g: bass.BassGpSimd):
            # issue input loads first so the transfers overlap the library
            # reload (the SDMA work needs no Q7 involvement once issued)
            for b, k in STREAMS:
                g.dma_start(idx_sb[b + k][:], idxs[b + k][:]).then_inc(io, 16)
            g.dma_start(csb[:], cnts[:]).then_inc(io, 16)
            g.load_library(mlp)
            g.wait_ge(io, 80)
            from contextlib import ExitStack as ES

            with ES() as rctx:
                regs = {
                    b + k: rctx.enter_context(g.register(f"r{b}{k}"))
                    for b, k in STREAMS
                }
                for i, (b, k) in enumerate(STREAMS):
                    g.reg_load(regs[b + k], csb[0:1, i : i + 1])
                for b, k in STREAMS:
                    cap, t, elem = _stream_dims(k)
                    src = tabs[b][:]
                    if k == "P":
                        src = src.rearrange("(a b) d -> a (b d)", b=2)
                    g.dma_gather(
                        tiles[b + k][:],
                        src,
                        idx_sb[b + k][:],
                        cap,
                        regs[b + k],
                        elem,
                        single_packet=False,
                    ).then_inc(gsem[b + k], 16)

        @block.sync
        def _(s: bass.BassEngine):
            for b, k in STREAMS:
                s.wait_ge(gsem[b + k], 16)
                s.dma_start(outs[b + k][:], tiles[b + k][:]).then_inc(wsem[b + k], 16)
            for b, k in STREAMS:
                s.wait_ge(wsem[b + k], 16)

    nc.compile()
    return nc


def get_program():
    if "nc" not in _prog_cache:
        _prog_cache["nc"] = _build_program()
    return _prog_cache["nc"]


def _slot_rows(cap, t):
    """Flattened [128*t, elem] device-buffer row for each gather list slot."""
    j = np.arange(cap, dtype=np.int64)
    return (j % 128) * t + j // 128


def _wrap_idx(seg, cap):
    """Pack a stream's int16 ids into the [128, cap/16] wrapped+replicated
    layout dma_gather expects (idx j at partition j%16, col j//16, replicated
    for the 8 Q7 cores), -1 padded."""
    pad = np.full(cap, -1, np.int16)
    pad[: seg.size] = seg
    wrapped = pad.reshape(cap // 16, 16).T  # [16, cap/16]
    return np.ascontiguousarray(np.tile(wrapped, (8, 1)))


def _route(cm, idx, n_weight_rows):
    """Deduplicated (bucket, local row) routing.

    Returns (ulocal, counts, inv, const_ids):
      ulocal    local table row per distinct slot, bucket-major, sorted
      counts    [16] distinct rows per bucket (bucket = slot*8 + core)
      inv       per-lookup index into the distinct-slot space
      const_ids row ids of X that form the compacted const table
    """
    const_rank = np.cumsum(cm) - 1
    var_pos = np.clip(np.cumsum(1 - cm) - 1, 0, n_weight_rows - 1)
    isc = cm[idx] > 0
    r = np.where(isc, const_rank[idx], var_pos[idx])
    bucket = (~isc).astype(np.int64) * NCORES + (r >> 15)
    key = bucket * SH + (r & (SH - 1))
    uniq, inv = np.unique(key, return_inverse=True)
    counts = np.bincount(uniq // SH, minlength=2 * NCORES)
    ulocal = uniq % SH
    const_ids = np.flatnonzero(cm > 0)
    return ulocal, counts, inv, const_ids


def _split_pairs(u):
    """Split sorted distinct rows into aligned pairs and singles.

    Returns (pairs, singles, ispaired, pos):
      pairs    sorted pair ids k where rows 2k and 2k+1 are both in u
      singles  sorted rows of u not covered by pairs
      ispaired bool per element of u
      pos      per element of u: its index within pairs (paired) or singles
    """
    pid = u >> 1
    eq = pid[:-1] == pid[1:]  # u[i], u[i+1] form an aligned pair
    ispaired = np.zeros(u.size, bool)
    ispaired[:-1] |= eq
    ispaired[1:] |= eq
    pairs = pid[:-1][eq]
    singles = u[~ispaired]
    pos = np.empty(u.size, np.int64)
    pos[ispaired] = np.searchsorted(pairs, pid[ispaired])
    pos[~ispaired] = np.arange(singles.size)
    return pairs, singles, ispaired, pos


def _kernel_numpy(X, cm, weight, idx):
    """Host fallback (used only if structural assumptions break)."""
    var_pos = np.clip(np.cumsum(1 - cm) - 1, 0, weight.shape[0] - 1)
    isc = cm[idx] > 0
    out = np.where(isc[:, None], X[idx], weight[var_pos[idx]])
    return out.astype(np.float32)


def kernel(X, const_mask, weight, index):
    X = np.ascontiguousarray(np.asarray(X), dtype=np.float32)
    weight = np.ascontiguousarray(np.asarray(weight), dtype=np.float32)
    cm = np.asarray(const_mask).astype(np.int64)
    idx = np.asarray(index).astype(np.int64)
    M = idx.shape[0]

    ulocal, counts, inv, const_ids = _route(cm, idx, weight.shape[0])
    starts = np.concatenate([[0], np.cumsum(counts)])

    # per-bucket pair/single split
    splits = [_split_pairs(ulocal[starts[b] : starts[b + 1]]) for b in range(16)]

    structural_ok = (
        X.shape == (524288, 128)
        and weight.shape == (262144, 128)
        and const_ids.size == NCORES * SH
        and weight.shape[0] == NCORES * SH
        and all(1 <= p.size <= CAP_P and 1 <= s.size <= CAP_S for p, s, _, _ in splits)
    )
    if not structural_ok:
        return _kernel_numpy(X, cm, weight, idx)

    Xe = X[const_ids]  # compacted const table [262144, 128]

    in_maps = []
    for c in range(NCORES):
        im = {
            "tabX": Xe[c * SH : (c + 1) * SH],
            "tabW": weight[c * SH : (c + 1) * SH],
        }
        cvec = np.empty(4, np.int32)
        for i, (b, k) in enumerate(STREAMS):
            bkt = (0 if b == "X" else NCORES) + c
            pairs, singles, _, _ = splits[bkt]
            ids = pairs if k == "P" else singles
            cap = _stream_dims(k)[0]
            im[f"idx{b}{k}"] = _wrap_idx(ids.astype(np.int16), cap)
            cvec[i] = ids.size
        im["cnts"] = np.ascontiguousarray(np.tile(cvec, (128, 1)))
        in_maps.append(im)

    nc = get_program()
    res = run_bass_kernel_spmd(nc, in_maps, core_ids=list(range(NCORES)))
    LAST["res"] = res

    # reassemble: distinct rows bucket-major, then expand duplicates per lookup
    srow_p = _slot_rows(CAP_P, TP)
    srow_s = _slot_rows(CAP_S, TS)
    allrows = np.empty((ulocal.size, D), np.float32)
    for c in range(NCORES):
        for b in ("X", "W"):
            bkt = (0 if b == "X" else NCORES) + c
            pairs, singles, ispaired, pos = splits[bkt]
            seg = slice(starts[bkt], starts[bkt + 1])
            u = ulocal[seg]
            arr = np.empty((u.size, D), np.float32)
            bufP = res.results[c][f"out{b}P"].reshape(128 * TP * 2, D)
            bufS = res.results[c][f"out{b}S"].reshape(128 * TS, D)
            arr[ispaired] = bufP[srow_p[pos[ispaired]] * 2 + (u[ispaired] & 1)]
            arr[~ispaired] = bufS[srow_s[pos[~ispaired]]]
            allrows[seg] = arr
    return allrows[inv]


# revision 18
# speedup vs baseline: 1.5205x; 1.0577x over previous
"""Embedding lookup (mixed const/trainable tables) on 8 Trainium2 NeuronCores.

Problem (full shapes, fp32):
    X          [524288, 128]   const table (only rows with const_mask==1 are read)
    const_mask [524288]        1 = const row (read from X), 0 = trainable row
    weight     [262144, 128]   trainable table, indexed by rank among mask==0 rows
    index      [262144]        lookup ids into the 524288-row id space
    out        [262144, 128]   out[i] = X[index[i]] if const else weight[var_pos[index[i]]]

Strategy (model parallel, deduplicated, pair-merged):
    - Host compacts X to its const rows (Xe) so both tables have 262144 rows;
      both are row-sharded over the 8 cores (32768 rows/core/table so local
      row ids fit dma_gather's int16 index format).
    - Each lookup routes to the owning (core, table) bucket. Each bucket is
      DEDUPLICATED (a distinct row is gathered once; duplicates expand in the
      host-side scatter) because GPSIMD descriptor generation (~8ns/row) is
      the kernel bottleneck, then split into
        * aligned PAIRS: rows (2k, 2k+1) both needed -> one 1024B descriptor
        * SINGLES: remaining rows -> one 512B descriptor
      cutting descriptor count another ~20%.
    - Actual pair/single counts are passed per core in a tiny `cnts` tensor
      and loaded into Q7 registers (num_idxs_reg), so the -1 tail padding in
      the index tiles costs nothing.
    - Device kernel per core: 4 dma_gather (GPSIMD SWDGE) HBM->SBUF streams,
      each followed by one large HWDGE write SBUF->HBM, overlapped.
    - Host scatters the gathered distinct rows back to all lookup positions.
"""

import numpy as np

import concourse.bass as bass
import concourse.bacc as bacc
import concourse.mybir as mybir
from concourse.bass_utils import run_bass_kernel_spmd
from concourse.library_config import mlp

NCORES = 8
D = 128             # feature dim (fp32) -> 512B rows
SH = 32768          # table rows per core per table (int16 gather index limit)

# Distinct rows per bucket: 16384 mean lookups hit 32768*(1-e^-0.5) ~= 12896
# distinct rows. Greedy run-cover pairing (any consecutive rows r, r+1 both
# present -> one 1024B descriptor at 512B stride) yields ~3640 pairs and
# ~5615 singles per bucket. Capacities are ~6-7 sigma above those means.
CAP_P = 4096        # pair slots per bucket   (TP = 32 tile columns of 1024B)
CAP_S = 6144        # single slots per bucket (TS = 48 tile columns of 512B)
TP = CAP_P // 128
TS = CAP_S // 128

# stream order: (bucket, kind) -> gather issue order; last write is the tail
# so end on a pair stream (smallest write).
STREAMS = (("X", "S"), ("X", "P"), ("W", "S"), ("W", "P"))

_prog_cache = {}
LAST = {}  # debug/profiling introspection for test harnesses


def _stream_dims(kind):
    if kind == "P":
        return CAP_P, TP, 2 * D  # 1024B elements (row pairs)
    return CAP_S, TS, D          # 512B elements


def _build_program():
    """Per-core SPMD bass program: 4 exact-count gather streams + 4 writes."""
    nc = bacc.Bacc("TRN2", target_bir_lowering=False)

    tabs = {
        "X": nc.dram_tensor("tabX", [SH, D], mybir.dt.float32, kind="ExternalInput"),
        "W": nc.dram_tensor("tabW", [SH, D], mybir.dt.float32, kind="ExternalInput"),
    }
    idxs, outs = {}, {}
    for b, k in STREAMS:
        cap, t, elem = _stream_dims(k)
        idxs[b + k] = nc.dram_tensor(
            f"idx{b}{k}", [128, cap // 16], mybir.dt.int16, kind="ExternalInput"
        )
        outs[b + k] = nc.dram_tensor(
            f"out{b}{k}", [128, t, elem], mybir.dt.float32, kind="ExternalOutput"
        )
    cnts = nc.dram_tensor("cnts", [128, 4], mybir.dt.int32, kind="ExternalInput")

    from contextlib import ExitStack

    with ExitStack() as ctx:
        # write-completion sems already guarantee all DMAs retired; skipping
        # the gpsimd dge_drain removes ~10us from the kernel tail
        block = ctx.enter_context(nc.Block(no_gpsimd_drain=True))
        idx_sb, tiles, gsem, wsem = {}, {}, {}, {}
        for b, k in STREAMS:
            cap, t, elem = _stream_dims(k)
            idx_sb[b + k] = ctx.enter_context(
                nc.sbuf_tensor(f"isb{b}{k}", [128, cap // 16], mybir.dt.int16)
            )
            tiles[b + k] = ctx.enter_context(
                nc.sbuf_tensor(f"tile{b}{k}", [128, t, elem], mybir.dt.float32)
            )
            gsem[b + k] = ctx.enter_context(nc.semaphore(f"g{b}{k}"))
            wsem[b + k] = ctx.enter_context(nc.semaphore(f"w{b}{k}"))
        csb = ctx.enter_context(nc.sbuf_tensor("csb", [128, 4], mybir.dt.int32))
        io = ctx.enter_context(nc.semaphore("io"))

        @block.gpsimd
        def _(g: bass.BassGpSimd):
            # issue input loads first so the transfers overlap the library
            # reload (the SDMA work needs no Q7 involvement once issued)
            for b, k in STREAMS:
                g.dma_start(idx_sb[b + k][:], idxs[b + k][:]).then_inc(io, 16)
            g.dma_start(csb[:], cnts[:]).then_inc(io, 16)
            g.load_library(mlp)
            g.wait_ge(io, 80)
            from contextlib import ExitStack as ES

            with ES() as rctx:
                regs = {
                    b + k: rctx.enter_context(g.register(f"r{b}{k}"))
                    for b, k in STREAMS
                }
                for i, (b, k) in enumerate(STREAMS):
                    g.reg_load(regs[b + k], csb[0:1, i : i + 1])
                for b, k in STREAMS:
                    cap, t, elem = _stream_dims(k)
                    if k == "P":
                        # overlapping view: row stride 128, element 256 ->
                        # idx r reads rows (r, r+1) as one 1024B descriptor
                        src = bass.AP(tabs[b], 0, [[D, SH - 1], [1, 2 * D]])
                        step = D
                    else:
                        src = tabs[b][:]
                        step = None
                    g.dma_gather(
                        tiles[b + k][:],
                        src,
                        idx_sb[b + k][:],
                        cap,
                        regs[b + k],
                        elem,
                        elem_step=step,
                        single_packet=False,
                    ).then_inc(gsem[b + k], 16)

        @block.sync
        def _(s: bass.BassEngine):
            for b, k in STREAMS:
                s.wait_ge(gsem[b + k], 16)
                s.dma_start(outs[b + k][:], tiles[b + k][:]).then_inc(wsem[b + k], 16)
            for b, k in STREAMS:
                s.wait_ge(wsem[b + k], 16)

    nc.compile()
    return nc


def get_program():
    if "nc" not in _prog_cache:
        _prog_cache["nc"] = _build_program()
    return _prog_cache["nc"]


def _slot_rows(cap, t):
    """Flattened [128*t, elem] device-buffer row for each gather list slot."""
    j = np.arange(cap, dtype=np.int64)
    return (j % 128) * t + j // 128


def _wrap_idx(seg, cap):
    """Pack a stream's int16 ids into the [128, cap/16] wrapped+replicated
    layout dma_gather expects (idx j at partition j%16, col j//16, replicated
    for the 8 Q7 cores), -1 padded."""
    pad = np.full(cap, -1, np.int16)
    pad[: seg.size] = seg
    wrapped = pad.reshape(cap // 16, 16).T  # [16, cap/16]
    return np.ascontiguousarray(np.tile(wrapped, (8, 1)))


def _route(cm, idx, n_weight_rows):
    """Deduplicated (bucket, local row) routing.

    Returns (ulocal, counts, inv, const_ids):
      ulocal    local table row per distinct slot, bucket-major, sorted
      counts    [16] distinct rows per bucket (bucket = slot*8 + core)
      inv       per-lookup index into the distinct-slot space
      const_ids row ids of X that form the compacted const table
    """
    const_rank = np.cumsum(cm) - 1
    var_pos = np.clip(np.cumsum(1 - cm) - 1, 0, n_weight_rows - 1)
    isc = cm[idx] > 0
    r = np.where(isc, const_rank[idx], var_pos[idx])
    bucket = (~isc).astype(np.int64) * NCORES + (r >> 15)
    key = bucket * SH + (r & (SH - 1))
    uniq, inv = np.unique(key, return_inverse=True)
    counts = np.bincount(uniq // SH, minlength=2 * NCORES)
    ulocal = uniq % SH
    const_ids = np.flatnonzero(cm > 0)
    return ulocal, counts, inv, const_ids


def _split_pairs(u):
    """Greedy run-cover of sorted distinct rows: consecutive rows pair up.

    A run of L consecutive rows becomes floor(L/2) pairs (gathered as one
    1024B element starting at the even offsets of the run) plus one single
    if L is odd.

    Returns (pairs, singles, ispaired, half, pos):
      pairs    start rows r of pairs (r, r+1), sorted
      singles  rows not covered by pairs, sorted
      ispaired bool per element of u
      half     0 = first row of its pair, 1 = second (paired elements only)
      pos      per element of u: its index within pairs resp. singles
    """
    n = u.size
    new_run = np.empty(n, bool)
    new_run[0] = True
    np.not_equal(np.diff(u), 1, out=new_run[1:])
    run_start = np.flatnonzero(new_run)
    run_id = np.cumsum(new_run) - 1
    off = np.arange(n) - run_start[run_id]
    is_pair_start = off % 2 == 0
    is_pair_start[:-1] &= run_id[:-1] == run_id[1:]
    is_pair_start[-1] = False  # last element has no successor to pair with
    second = np.zeros(n, bool)
    second[1:] = is_pair_start[:-1]
    ispaired = is_pair_start | second
    pairs = u[is_pair_start]
    singles = u[~ispaired]
    pair_rank = np.cumsum(is_pair_start) - 1
    single_rank = np.cumsum(~ispaired) - 1
    pos = np.empty(n, np.int64)
    pos[is_pair_start] = pair_rank[is_pair_start]
    pos[second] = pair_rank[np.flatnonzero(second) - 1]
    pos[~ispaired] = single_rank[~ispaired]
    return pairs, singles, ispaired, second.astype(np.int64), pos


def _kernel_numpy(X, cm, weight, idx):
    """Host fallback (used only if structural assumptions break)."""
    var_pos = np.clip(np.cumsum(1 - cm) - 1, 0, weight.shape[0] - 1)
    isc = cm[idx] > 0
    out = np.where(isc[:, None], X[idx], weight[var_pos[idx]])
    return out.astype(np.float32)


def kernel(X, const_mask, weight, index):
    X = np.ascontiguousarray(np.asarray(X), dtype=np.float32)
    weight = np.ascontiguousarray(np.asarray(weight), dtype=np.float32)
    cm = np.asarray(const_mask).astype(np.int64)
    idx = np.asarray(index).astype(np.int64)
    M = idx.shape[0]

    ulocal, counts, inv, const_ids = _route(cm, idx, weight.shape[0])
    starts = np.concatenate([[0], np.cumsum(counts)])

    # per-bucket pair/single split
    splits = [_split_pairs(ulocal[starts[b] : starts[b + 1]]) for b in range(16)]

    structural_ok = (
        X.shape == (524288, 128)
        and weight.shape == (262144, 128)
        and const_ids.size == NCORES * SH
        and weight.shape[0] == NCORES * SH
        and all(
            1 <= p.size <= CAP_P and 1 <= s.size <= CAP_S
            for p, s, _, _, _ in splits
        )
    )
    if not structural_ok:
        return _kernel_numpy(X, cm, weight, idx)

    Xe = X[const_ids]  # compacted const table [262144, 128]

    in_maps = []
    for c in range(NCORES):
        im = {
            "tabX": Xe[c * SH : (c + 1) * SH],
            "tabW": weight[c * SH : (c + 1) * SH],
        }
        cvec = np.empty(4, np.int32)
        for i, (b, k) in enumerate(STREAMS):
            bkt = (0 if b == "X" else NCORES) + c
            pairs, singles, _, _, _ = splits[bkt]
            ids = pairs if k == "P" else singles
            cap = _stream_dims(k)[0]
            im[f"idx{b}{k}"] = _wrap_idx(ids.astype(np.int16), cap)
            cvec[i] = ids.size
        im["cnts"] = np.ascontiguousarray(np.tile(cvec, (128, 1)))
        in_maps.append(im)

    nc = get_program()
    res = run_bass_kernel_spmd(nc, in_maps, core_ids=list(range(NCORES)))
    LAST["res"] = res

    # reassemble: distinct rows bucket-major, then expand duplicates per lookup
    srow_p = _slot_rows(CAP_P, TP)
    srow_s = _slot_rows(CAP_S, TS)
    allrows = np.empty((ulocal.size, D), np.float32)
    for c in range(NCORES):
        for b in ("X", "W"):
            bkt = (0 if b == "X" else NCORES) + c
            pairs, singles, ispaired, half, pos = splits[bkt]
            seg = slice(starts[bkt], starts[bkt + 1])
            arr = np.empty((ispaired.size, D), np.float32)
            bufP = res.results[c][f"out{b}P"].reshape(128 * TP * 2, D)
            bufS = res.results[c][f"out{b}S"].reshape(128 * TS, D)
            arr[ispaired] = bufP[srow_p[pos[ispaired]] * 2 + half[ispaired]]
            arr[~ispaired] = bufS[srow_s[pos[~ispaired]]]
            allrows[seg] = arr
    return allrows[inv]


# revision 22
# speedup vs baseline: 1.6147x; 1.0619x over previous
"""Embedding lookup (mixed const/trainable tables) on 8 Trainium2 NeuronCores.

Problem (full shapes, fp32):
    X          [524288, 128]   const table (only rows with const_mask==1 are read)
    const_mask [524288]        1 = const row (read from X), 0 = trainable row
    weight     [262144, 128]   trainable table, indexed by rank among mask==0 rows
    index      [262144]        lookup ids into the 524288-row id space
    out        [262144, 128]   out[i] = X[index[i]] if const else weight[var_pos[index[i]]]

Strategy (model parallel, deduplicated, pair-merged):
    - Host compacts X to its const rows (Xe) so both tables have 262144 rows;
      both are row-sharded over the 8 cores (32768 rows/core/table so local
      row ids fit dma_gather's int16 index format).
    - Each lookup routes to the owning (core, table) bucket. Each bucket is
      DEDUPLICATED (a distinct row is gathered once; duplicates expand in the
      host-side scatter) because GPSIMD descriptor generation (~8ns/row) is
      the kernel bottleneck, then split into
        * aligned PAIRS: rows (2k, 2k+1) both needed -> one 1024B descriptor
        * SINGLES: remaining rows -> one 512B descriptor
      cutting descriptor count another ~20%.
    - Actual pair/single counts are passed per core in a tiny `cnts` tensor
      and loaded into Q7 registers (num_idxs_reg), so the -1 tail padding in
      the index tiles costs nothing.
    - Device kernel per core: 4 dma_gather (GPSIMD SWDGE) HBM->SBUF streams,
      each followed by one large HWDGE write SBUF->HBM, overlapped.
    - Host scatters the gathered distinct rows back to all lookup positions.
"""

import numpy as np

import concourse.bass as bass
import concourse.bacc as bacc
import concourse.mybir as mybir
from concourse.bass_utils import run_bass_kernel_spmd
from concourse.library_config import mlp

NCORES = 8
D = 128             # feature dim (fp32) -> 512B rows
SH = 32768          # table rows per core per table (int16 gather index limit)

# Distinct rows per bucket: 16384 mean lookups hit 32768*(1-e^-0.5) ~= 12896
# distinct rows. Greedy run-cover pairing (any consecutive rows r, r+1 both
# present -> one 1024B descriptor at 512B stride) yields ~3640 pairs and
# ~5615 singles per bucket. Capacities are ~6-7 sigma above those means.
CAP_P = 4096        # pair slots per bucket   (TP = 32 tile columns of 1024B)
CAP_S = 6144        # single slots per bucket (TS = 48 tile columns of 512B)
TP = CAP_P // 128
TS = CAP_S // 128

# Gather streams in issue order: (name, bucket, kind, cap, offset-into-list).
# The only write that cannot hide under a later gather is the LAST one, so
# the W singles are split and the kernel ends on a small 1.3MB write.
CAP_S1 = 3584
CAP_S2 = CAP_S - CAP_S1  # 2560
STREAMS = (
    ("XP", "X", "P", CAP_P, 0),
    ("XS", "X", "S", CAP_S, 0),
    ("WP", "W", "P", CAP_P, 0),
    ("WS1", "W", "S", CAP_S1, 0),
    ("WS2", "W", "S", CAP_S2, CAP_S1),
)

_prog_cache = {}
LAST = {}  # debug/profiling introspection for test harnesses


def _elem(kind):
    return 2 * D if kind == "P" else D


def _build_program():
    """Per-core SPMD bass program: exact-count gather streams + writes."""
    nc = bacc.Bacc("TRN2", target_bir_lowering=False)

    tabs = {
        "X": nc.dram_tensor("tabX", [SH, D], mybir.dt.float32, kind="ExternalInput"),
        "W": nc.dram_tensor("tabW", [SH, D], mybir.dt.float32, kind="ExternalInput"),
    }
    idxs, outs = {}, {}
    for nm, b, k, cap, off in STREAMS:
        idxs[nm] = nc.dram_tensor(
            f"idx{nm}", [128, cap // 16], mybir.dt.int16, kind="ExternalInput"
        )
        outs[nm] = nc.dram_tensor(
            f"out{nm}", [128, cap // 128, _elem(k)], mybir.dt.float32,
            kind="ExternalOutput",
        )
    cnts = nc.dram_tensor(
        "cnts", [128, len(STREAMS)], mybir.dt.int32, kind="ExternalInput"
    )

    from contextlib import ExitStack

    with ExitStack() as ctx:
        # write-completion sems already guarantee all DMAs retired; skipping
        # the gpsimd dge_drain removes ~10us from the kernel tail
        block = ctx.enter_context(nc.Block(no_gpsimd_drain=True))
        idx_sb, tiles, gsem, wsem = {}, {}, {}, {}
        for nm, b, k, cap, off in STREAMS:
            idx_sb[nm] = ctx.enter_context(
                nc.sbuf_tensor(f"isb{nm}", [128, cap // 16], mybir.dt.int16)
            )
            tiles[nm] = ctx.enter_context(
                nc.sbuf_tensor(f"tile{nm}", [128, cap // 128, _elem(k)],
                               mybir.dt.float32)
            )
            gsem[nm] = ctx.enter_context(nc.semaphore(f"g{nm}"))
            wsem[nm] = ctx.enter_context(nc.semaphore(f"w{nm}"))
        csb = ctx.enter_context(
            nc.sbuf_tensor("csb", [128, len(STREAMS)], mybir.dt.int32)
        )
        io = ctx.enter_context(nc.semaphore("io"))
        n_in = 16 * (len(STREAMS) + 1)

        @block.gpsimd
        def _(g: bass.BassGpSimd):
            # issue input loads first so the transfers overlap the library
            # reload (the SDMA work needs no Q7 involvement once issued)
            for nm, *_ in STREAMS:
                g.dma_start(idx_sb[nm][:], idxs[nm][:]).then_inc(io, 16)
            g.dma_start(csb[:], cnts[:]).then_inc(io, 16)
            g.load_library(mlp)
            g.wait_ge(io, n_in)
            from contextlib import ExitStack as ES

            with ES() as rctx:
                regs = {
                    nm: rctx.enter_context(g.register(f"r{nm}"))
                    for nm, *_ in STREAMS
                }
                for i, (nm, *_) in enumerate(STREAMS):
                    g.reg_load(regs[nm], csb[0:1, i : i + 1])
                for nm, b, k, cap, off in STREAMS:
                    if k == "P":
                        # overlapping view: row stride 128, element 256 ->
                        # idx r reads rows (r, r+1) as one 1024B descriptor
                        src = bass.AP(tabs[b], 0, [[D, SH - 1], [1, 2 * D]])
                        step = D
                    else:
                        src = tabs[b][:]
                        step = None
                    g.dma_gather(
                        tiles[nm][:],
                        src,
                        idx_sb[nm][:],
                        cap,
                        regs[nm],
                        _elem(k),
                        elem_step=step,
                        single_packet=False,
                    ).then_inc(gsem[nm], 16)

        @block.sync
        def _(s: bass.BassEngine):
            for nm, *_ in STREAMS:
                s.wait_ge(gsem[nm], 16)
                s.dma_start(outs[nm][:], tiles[nm][:]).then_inc(wsem[nm], 16)
            for nm, *_ in STREAMS:
                s.wait_ge(wsem[nm], 16)

    nc.compile()
    return nc


def get_program():
    if "nc" not in _prog_cache:
        _prog_cache["nc"] = _build_program()
    return _prog_cache["nc"]


def _slot_rows(cap, t):
    """Flattened [128*t, elem] device-buffer row for each gather list slot."""
    j = np.arange(cap, dtype=np.int64)
    return (j % 128) * t + j // 128


def _wrap_idx(seg, cap):
    """Pack a stream's int16 ids into the [128, cap/16] wrapped+replicated
    layout dma_gather expects (idx j at partition j%16, col j//16, replicated
    for the 8 Q7 cores), -1 padded."""
    pad = np.full(cap, -1, np.int16)
    pad[: seg.size] = seg
    wrapped = pad.reshape(cap // 16, 16).T  # [16, cap/16]
    return np.ascontiguousarray(np.tile(wrapped, (8, 1)))


def _route(cm, idx, n_weight_rows):
    """Deduplicated (bucket, local row) routing.

    Returns (ulocal, counts, inv, const_ids):
      ulocal    local table row per distinct slot, bucket-major, sorted
      counts    [16] distinct rows per bucket (bucket = slot*8 + core)
      inv       per-lookup index into the distinct-slot space
      const_ids row ids of X that form the compacted const table
    """
    const_rank = np.cumsum(cm) - 1
    var_pos = np.clip(np.cumsum(1 - cm) - 1, 0, n_weight_rows - 1)
    isc = cm[idx] > 0
    r = np.where(isc, const_rank[idx], var_pos[idx])
    bucket = (~isc).astype(np.int64) * NCORES + (r >> 15)
    key = bucket * SH + (r & (SH - 1))
    uniq, inv = np.unique(key, return_inverse=True)
    counts = np.bincount(uniq // SH, minlength=2 * NCORES)
    ulocal = uniq % SH
    const_ids = np.flatnonzero(cm > 0)
    return ulocal, counts, inv, const_ids


def _split_pairs(u):
    """Greedy run-cover of sorted distinct rows: consecutive rows pair up.

    A run of L consecutive rows becomes floor(L/2) pairs (gathered as one
    1024B element starting at the even offsets of the run) plus one single
    if L is odd.

    Returns (pairs, singles, ispaired, half, pos):
      pairs    start rows r of pairs (r, r+1), sorted
      singles  rows not covered by pairs, sorted
      ispaired bool per element of u
      half     0 = first row of its pair, 1 = second (paired elements only)
      pos      per element of u: its index within pairs resp. singles
    """
    n = u.size
    new_run = np.empty(n, bool)
    new_run[0] = True
    np.not_equal(np.diff(u), 1, out=new_run[1:])
    run_start = np.flatnonzero(new_run)
    run_id = np.cumsum(new_run) - 1
    off = np.arange(n) - run_start[run_id]
    is_pair_start = off % 2 == 0
    is_pair_start[:-1] &= run_id[:-1] == run_id[1:]
    is_pair_start[-1] = False  # last element has no successor to pair with
    second = np.zeros(n, bool)
    second[1:] = is_pair_start[:-1]
    ispaired = is_pair_start | second
    pairs = u[is_pair_start]
    singles = u[~ispaired]
    pair_rank = np.cumsum(is_pair_start) - 1
    single_rank = np.cumsum(~ispaired) - 1
    pos = np.empty(n, np.int64)
    pos[is_pair_start] = pair_rank[is_pair_start]
    pos[second] = pair_rank[np.flatnonzero(second) - 1]
    pos[~ispaired] = single_rank[~ispaired]
    return pairs, singles, ispaired, second.astype(np.int64), pos


def _kernel_numpy(X, cm, weight, idx):
    """Host fallback (used only if structural assumptions break)."""
    var_pos = np.clip(np.cumsum(1 - cm) - 1, 0, weight.shape[0] - 1)
    isc = cm[idx] > 0
    out = np.where(isc[:, None], X[idx], weight[var_pos[idx]])
    return out.astype(np.float32)


def kernel(X, const_mask, weight, index):
    X = np.ascontiguousarray(np.asarray(X), dtype=np.float32)
    weight = np.ascontiguousarray(np.asarray(weight), dtype=np.float32)
    cm = np.asarray(const_mask).astype(np.int64)
    idx = np.asarray(index).astype(np.int64)
    M = idx.shape[0]

    ulocal, counts, inv, const_ids = _route(cm, idx, weight.shape[0])
    starts = np.concatenate([[0], np.cumsum(counts)])

    # per-bucket pair/single split
    splits = [_split_pairs(ulocal[starts[b] : starts[b + 1]]) for b in range(16)]

    # per (bucket, kind): list of (name, cap, off) covering the id list
    segs = {}
    for nm, b, k, cap, off in STREAMS:
        segs.setdefault((b, k), []).append((nm, cap, off))

    def _cap_ok(bkt):
        pairs, singles, _, _, _ = splits[bkt]
        b = "X" if bkt < NCORES else "W"
        if not 1 <= pairs.size <= CAP_P:
            return False
        # every split segment must be non-empty (a zero-count gather is
        # undefined) and the full list must fit the combined capacity
        lo = segs[(b, "S")][-1][2]  # offset of the last segment
        return lo < singles.size <= CAP_S

    structural_ok = (
        X.shape == (524288, 128)
        and weight.shape == (262144, 128)
        and const_ids.size == NCORES * SH
        and weight.shape[0] == NCORES * SH
        and all(_cap_ok(bkt) for bkt in range(2 * NCORES))
    )
    if not structural_ok:
        return _kernel_numpy(X, cm, weight, idx)

    Xe = X[const_ids]  # compacted const table [262144, 128]

    in_maps = []
    for c in range(NCORES):
        im = {
            "tabX": Xe[c * SH : (c + 1) * SH],
            "tabW": weight[c * SH : (c + 1) * SH],
        }
        cvec = np.empty(len(STREAMS), np.int32)
        for i, (nm, b, k, cap, off) in enumerate(STREAMS):
            bkt = (0 if b == "X" else NCORES) + c
            pairs, singles, _, _, _ = splits[bkt]
            ids = (pairs if k == "P" else singles)[off : off + cap]
            im[f"idx{nm}"] = _wrap_idx(ids.astype(np.int16), cap)
            cvec[i] = ids.size
        im["cnts"] = np.ascontiguousarray(np.tile(cvec, (128, 1)))
        in_maps.append(im)

    nc = get_program()
    res = run_bass_kernel_spmd(nc, in_maps, core_ids=list(range(NCORES)))
    LAST["res"] = res

    # reassemble: distinct rows bucket-major, then expand duplicates per lookup
    allrows = np.empty((ulocal.size, D), np.float32)
    for c in range(NCORES):
        for b in ("X", "W"):
            bkt = (0 if b == "X" else NCORES) + c
            pairs, singles, ispaired, half, pos = splits[bkt]
            seg = slice(starts[bkt], starts[bkt + 1])
            arr = np.empty((ispaired.size, D), np.float32)
            # pairs: single stream per bucket; flat row = slot*2 + half
            (pnm, pcap, _), = segs[(b, "P")]
            bufP = res.results[c][f"out{pnm}"].reshape(-1, D)
            sp = _slot_rows(pcap, pcap // 128)
            arr[ispaired] = bufP[sp[pos[ispaired]] * 2 + half[ispaired]]
            # singles: possibly split across streams by offset
            spos = pos[~ispaired]
            sarr = np.empty((spos.size, D), np.float32)
            for snm, scap, soff in segs[(b, "S")]:
                bufS = res.results[c][f"out{snm}"].reshape(-1, D)
                ss = _slot_rows(scap, scap // 128)
                m = (spos >= soff) & (spos < soff + scap)
                sarr[m] = bufS[ss[spos[m] - soff]]
            arr[~ispaired] = sarr
            allrows[seg] = arr
    return allrows[inv]


# revision 24
# speedup vs baseline: 1.7403x; 1.0778x over previous
"""Embedding lookup (mixed const/trainable tables) on 8 Trainium2 NeuronCores.

Problem (full shapes, fp32):
    X          [524288, 128]   const table (only rows with const_mask==1 are read)
    const_mask [524288]        1 = const row (read from X), 0 = trainable row
    weight     [262144, 128]   trainable table, indexed by rank among mask==0 rows
    index      [262144]        lookup ids into the 524288-row id space
    out        [262144, 128]   out[i] = X[index[i]] if const else weight[var_pos[index[i]]]

Strategy (model parallel, deduplicated, run-covered):
    - Host compacts X to its const rows (Xe) so both tables have 262144 rows;
      both are row-sharded over the 8 cores (32768 rows/core/table so local
      row ids fit dma_gather's int16 index format).
    - Each lookup routes to the owning (core, table) bucket. Buckets are
      DEDUPLICATED (a distinct row is gathered once; duplicates expand in the
      host-side scatter) because GPSIMD descriptor generation (~8-9ns per
      descriptor) is the kernel bottleneck — not bandwidth.
    - Each bucket's sorted distinct rows are covered by three descriptor
      tiers using dma_gather's elem_step (row stride) < elem_size overlap:
        * QUADS  idx r -> rows r..r+3 as one 2048B descriptor
        * PAIRS  idx r -> rows r,r+1  as one 1024B descriptor
        * SINGLES idx r -> row r      as one  512B descriptor
      A run of L consecutive needed rows takes L//4 quads plus one tail
      element (L%4 = 3 rounds UP to a quad, reading one junk row — one
      descriptor is worth more than 512B of bandwidth here).
    - Exact tier counts ride in a tiny `cnts` input and are loaded into Q7
      registers (num_idxs_reg), so -1 index padding costs nothing.
    - Device kernel per core: 7 dma_gather (GPSIMD SWDGE) HBM->SBUF streams,
      each followed by one large HWDGE write SBUF->HBM, overlapped; the W
      singles are split so the kernel tail is one small write.
    - Host scatters the gathered distinct rows back to all lookup positions.
"""

import numpy as np

import concourse.bass as bass
import concourse.bacc as bacc
import concourse.mybir as mybir
from concourse.bass_utils import run_bass_kernel_spmd
from concourse.library_config import mlp

NCORES = 8
D = 128             # feature dim (fp32) -> 512B rows
SH = 32768          # table rows per core per table (int16 gather index limit)

# Distinct rows per bucket: 16384 mean lookups hit 32768*(1-e^-0.5) ~= 12896
# distinct rows in ~7820 runs -> ~1270 quads, ~1930 pairs, ~4875 singles.
# Capacities are ~6-7 sigma above those means.
CAP_Q = 1536
CAP_P = 2176
CAP_S = 5376
CAP_S1 = 2944       # W singles split so the last write is small
CAP_S2 = CAP_S - CAP_S1

# rows covered per descriptor by tier
TIER_ROWS = {"Q": 4, "P": 2, "S": 1}

# Gather streams in issue order: (name, bucket, tier, cap, offset-into-list).
STREAMS = (
    ("XQ", "X", "Q", CAP_Q, 0),
    ("XP", "X", "P", CAP_P, 0),
    ("XS", "X", "S", CAP_S, 0),
    ("WQ", "W", "Q", CAP_Q, 0),
    ("WP", "W", "P", CAP_P, 0),
    ("WS1", "W", "S", CAP_S1, 0),
    ("WS2", "W", "S", CAP_S2, CAP_S1),
)

_prog_cache = {}
LAST = {}  # debug/profiling introspection for test harnesses


def _elem(tier):
    return TIER_ROWS[tier] * D


def _build_program():
    """Per-core SPMD bass program: exact-count gather streams + writes."""
    nc = bacc.Bacc("TRN2", target_bir_lowering=False)

    tabs = {
        "X": nc.dram_tensor("tabX", [SH, D], mybir.dt.float32, kind="ExternalInput"),
        "W": nc.dram_tensor("tabW", [SH, D], mybir.dt.float32, kind="ExternalInput"),
    }
    idxs, outs = {}, {}
    for nm, b, k, cap, off in STREAMS:
        idxs[nm] = nc.dram_tensor(
            f"idx{nm}", [128, cap // 16], mybir.dt.int16, kind="ExternalInput"
        )
        outs[nm] = nc.dram_tensor(
            f"out{nm}", [128, cap // 128, _elem(k)], mybir.dt.float32,
            kind="ExternalOutput",
        )
    cnts = nc.dram_tensor(
        "cnts", [128, len(STREAMS)], mybir.dt.int32, kind="ExternalInput"
    )

    from contextlib import ExitStack

    with ExitStack() as ctx:
        # write-completion sems already guarantee all DMAs retired; skipping
        # the gpsimd dge_drain removes ~10us from the kernel tail
        block = ctx.enter_context(nc.Block(no_gpsimd_drain=True))
        idx_sb, tiles, gsem, wsem = {}, {}, {}, {}
        for nm, b, k, cap, off in STREAMS:
            idx_sb[nm] = ctx.enter_context(
                nc.sbuf_tensor(f"isb{nm}", [128, cap // 16], mybir.dt.int16)
            )
            tiles[nm] = ctx.enter_context(
                nc.sbuf_tensor(f"tile{nm}", [128, cap // 128, _elem(k)],
                               mybir.dt.float32)
            )
            gsem[nm] = ctx.enter_context(nc.semaphore(f"g{nm}"))
            wsem[nm] = ctx.enter_context(nc.semaphore(f"w{nm}"))
        csb = ctx.enter_context(
            nc.sbuf_tensor("csb", [128, len(STREAMS)], mybir.dt.int32)
        )
        io = ctx.enter_context(nc.semaphore("io"))
        n_in = 16 * (len(STREAMS) + 1)

        @block.gpsimd
        def _(g: bass.BassGpSimd):
            # issue input loads first so the transfers overlap the library
            # reload (the SDMA work needs no Q7 involvement once issued)
            for nm, *_ in STREAMS:
                g.dma_start(idx_sb[nm][:], idxs[nm][:]).then_inc(io, 16)
            g.dma_start(csb[:], cnts[:]).then_inc(io, 16)
            g.load_library(mlp)
            g.wait_ge(io, n_in)
            from contextlib import ExitStack as ES

            with ES() as rctx:
                regs = {
                    nm: rctx.enter_context(g.register(f"r{nm}"))
                    for nm, *_ in STREAMS
                }
                for i, (nm, *_) in enumerate(STREAMS):
                    g.reg_load(regs[nm], csb[0:1, i : i + 1])
                for nm, b, k, cap, off in STREAMS:
                    rows = TIER_ROWS[k]
                    if rows > 1:
                        # overlapping view: row stride D, element rows*D ->
                        # idx r reads rows r..r+rows-1 as one descriptor
                        src = bass.AP(
                            tabs[b], 0, [[D, SH - (rows - 1)], [1, rows * D]]
                        )
                        step = D
                    else:
                        src = tabs[b][:]
                        step = None
                    g.dma_gather(
                        tiles[nm][:],
                        src,
                        idx_sb[nm][:],
                        cap,
                        regs[nm],
                        _elem(k),
                        elem_step=step,
                        single_packet=False,
                    ).then_inc(gsem[nm], 16)

        @block.sync
        def _(s: bass.BassEngine):
            for nm, *_ in STREAMS:
                s.wait_ge(gsem[nm], 16)
                s.dma_start(outs[nm][:], tiles[nm][:]).then_inc(wsem[nm], 16)
            for nm, *_ in STREAMS:
                s.wait_ge(wsem[nm], 16)

    nc.compile()
    return nc


def get_program():
    if "nc" not in _prog_cache:
        _prog_cache["nc"] = _build_program()
    return _prog_cache["nc"]


def _slot_rows(cap):
    """Flattened [128*(cap/128), elem] device-buffer row per gather slot."""
    j = np.arange(cap, dtype=np.int64)
    return (j % 128) * (cap // 128) + j // 128


def _wrap_idx(seg, cap):
    """Pack a stream's int16 ids into the [128, cap/16] wrapped+replicated
    layout dma_gather expects (idx j at partition j%16, col j//16, replicated
    for the 8 Q7 cores), -1 padded."""
    pad = np.full(cap, -1, np.int16)
    pad[: seg.size] = seg
    wrapped = pad.reshape(cap // 16, 16).T  # [16, cap/16]
    return np.ascontiguousarray(np.tile(wrapped, (8, 1)))


def _route(cm, idx, n_weight_rows):
    """Deduplicated (bucket, local row) routing.

    Returns (ulocal, counts, inv, const_ids):
      ulocal    local table row per distinct slot, bucket-major, sorted
      counts    [16] distinct rows per bucket (bucket = slot*8 + core)
      inv       per-lookup index into the distinct-slot space
      const_ids row ids of X that form the compacted const table
    """
    const_rank = np.cumsum(cm) - 1
    var_pos = np.clip(np.cumsum(1 - cm) - 1, 0, n_weight_rows - 1)
    isc = cm[idx] > 0
    r = np.where(isc, const_rank[idx], var_pos[idx])
    bucket = (~isc).astype(np.int64) * NCORES + (r >> 15)
    key = bucket * SH + (r & (SH - 1))
    uniq, inv = np.unique(key, return_inverse=True)
    counts = np.bincount(uniq // SH, minlength=2 * NCORES)
    ulocal = uniq % SH
    const_ids = np.flatnonzero(cm > 0)
    return ulocal, counts, inv, const_ids


def _cover_runs(u):
    """Cover sorted distinct rows with quad/pair/single descriptors.

    Each run of L consecutive rows takes L//4 quads; the tail (L%4) becomes a
    waste-quad (L%4==3, reads one junk row), a pair, or a single. A tail quad
    that would read past the table falls back to pair+single.

    Returns (tiers, elmap) where
      tiers = {"Q": start rows, "P": start rows, "S": rows} (each sorted)
      elmap = (tier_code, start, off) per element of u: tier 0/1/2 = Q/P/S,
              `start` the covering descriptor's start row, `off` the row
              offset inside the descriptor.
    """
    n = u.size
    new_run = np.empty(n, bool)
    new_run[0] = True
    np.not_equal(np.diff(u), 1, out=new_run[1:])
    rstart = np.flatnonzero(new_run)          # index into u of run starts
    run_id = np.cumsum(new_run) - 1
    L = np.diff(np.append(rstart, n))
    v = u[rstart]
    nq = L // 4
    rem = L % 4
    tail = v + 4 * nq                          # start row of the tail element
    extraq = (rem == 3) & (tail <= SH - 4)     # waste-quad fits in the table
    fb3 = (rem == 3) & ~extraq                 # boundary fallback pair+single

    totq = int(nq.sum())
    base = np.repeat(v, nq)
    first = np.repeat(np.cumsum(nq) - nq, nq)
    quads_main = base + 4 * (np.arange(totq) - first)
    quads = np.sort(np.concatenate([quads_main, tail[extraq]]))
    pairs = np.sort(np.concatenate([tail[rem == 2], tail[fb3]]))
    singles = np.sort(np.concatenate([tail[rem == 1], tail[fb3] + 2]))

    # per-element mapping
    o = np.arange(n) - rstart[run_id]
    rnq = nq[run_id]
    in_main = o // 4 < rnq
    t = o - 4 * rnq                            # tail offset (valid if not main)
    rrem = rem[run_id]
    rextraq = extraq[run_id]
    tier = np.empty(n, np.int8)
    start = np.empty(n, np.int64)
    off = np.empty(n, np.int64)
    # main quads
    tier[in_main] = 0
    start[in_main] = u[in_main] - o[in_main] % 4
    off[in_main] = o[in_main] % 4
    tl = ~in_main
    # tail: waste quad
    m = tl & rextraq
    tier[m] = 0
    start[m] = u[m] - t[m]
    off[m] = t[m]
    # tail: rem 2 pair, or fallback3 pair part (t in 0,1)
    m = tl & ((rrem == 2) | ((rrem == 3) & ~rextraq & (t < 2)))
    tier[m] = 1
    start[m] = u[m] - t[m]
    off[m] = t[m]
    # tail: rem 1 single, or fallback3 single part (t == 2)
    m = tl & ((rrem == 1) | ((rrem == 3) & ~rextraq & (t == 2)))
    tier[m] = 2
    start[m] = u[m]
    off[m] = 0
    return {"Q": quads, "P": pairs, "S": singles}, (tier, start, off)


def _kernel_numpy(X, cm, weight, idx):
    """Host fallback (used only if structural assumptions break)."""
    var_pos = np.clip(np.cumsum(1 - cm) - 1, 0, weight.shape[0] - 1)
    isc = cm[idx] > 0
    out = np.where(isc[:, None], X[idx], weight[var_pos[idx]])
    return out.astype(np.float32)


def kernel(X, const_mask, weight, index):
    X = np.ascontiguousarray(np.asarray(X), dtype=np.float32)
    weight = np.ascontiguousarray(np.asarray(weight), dtype=np.float32)
    cm = np.asarray(const_mask).astype(np.int64)
    idx = np.asarray(index).astype(np.int64)
    M = idx.shape[0]

    ulocal, counts, inv, const_ids = _route(cm, idx, weight.shape[0])
    starts = np.concatenate([[0], np.cumsum(counts)])
    covers = [_cover_runs(ulocal[starts[b] : starts[b + 1]]) for b in range(16)]

    # per (bucket, tier): stream segments covering the id list
    segs = {}
    for nm, b, k, cap, off in STREAMS:
        segs.setdefault((b, k), []).append((nm, cap, off))

    def _cap_ok(bkt):
        tiers, _ = covers[bkt]
        b = "X" if bkt < NCORES else "W"
        for k in ("Q", "P", "S"):
            lst = segs[(b, k)]
            total_cap = sum(cap for _, cap, _ in lst)
            last_off = lst[-1][2]
            # every split segment must be non-empty (a zero-count gather is
            # undefined) and the full list must fit the combined capacity
            if not last_off < tiers[k].size <= total_cap:
                return False
        return True

    structural_ok = (
        X.shape == (524288, 128)
        and weight.shape == (262144, 128)
        and const_ids.size == NCORES * SH
        and weight.shape[0] == NCORES * SH
        and all(_cap_ok(bkt) for bkt in range(2 * NCORES))
    )
    if not structural_ok:
        return _kernel_numpy(X, cm, weight, idx)

    Xe = X[const_ids]  # compacted const table [262144, 128]

    in_maps = []
    for c in range(NCORES):
        im = {
            "tabX": Xe[c * SH : (c + 1) * SH],
            "tabW": weight[c * SH : (c + 1) * SH],
        }
        cvec = np.empty(len(STREAMS), np.int32)
        for i, (nm, b, k, cap, off) in enumerate(STREAMS):
            bkt = (0 if b == "X" else NCORES) + c
            ids = covers[bkt][0][k][off : off + cap]
            im[f"idx{nm}"] = _wrap_idx(ids.astype(np.int16), cap)
            cvec[i] = ids.size
        im["cnts"] = np.ascontiguousarray(np.tile(cvec, (128, 1)))
        in_maps.append(im)

    nc = get_program()
    res = run_bass_kernel_spmd(nc, in_maps, core_ids=list(range(NCORES)))
    LAST["res"] = res

    # reassemble: distinct rows bucket-major, then expand duplicates per lookup
    allrows = np.empty((ulocal.size, D), np.float32)
    for c in range(NCORES):
        for b in ("X", "W"):
            bkt = (0 if b == "X" else NCORES) + c
            tiers, (tier, start, off) = covers[bkt]
            seg = slice(starts[bkt], starts[bkt + 1])
            arr = np.empty((tier.size, D), np.float32)
            for code, k in ((0, "Q"), (1, "P"), (2, "S")):
                rows = TIER_ROWS[k]
                m = tier == code
                pos = np.searchsorted(tiers[k], start[m])
                offm = off[m]
                vals = np.empty((pos.size, D), np.float32)
                for snm, scap, soff in segs[(b, k)]:
                    buf = res.results[c][f"out{snm}"].reshape(-1, D)
                    sr = _slot_rows(scap)
                    sm = (pos >= soff) & (pos < soff + scap)
                    vals[sm] = buf[sr[pos[sm] - soff] * rows + offm[sm]]
                arr[m] = vals
            allrows[seg] = arr
    return allrows[inv]
